# revision 31
# baseline (speedup 1.0000x reference)
"""Trainium2 Bass kernel for CombinedLSTMWithStatic2Hop.

Model: per-node LSTM over T timesteps + static encoder -> fusion -> 2x SAGEConv
(mean aggregation) -> linear head.

Sharding: B*N = 8000 nodes split into 1000 contiguous nodes per core (8 cores).
Each graph (2000 nodes) spans a core pair; SAGE aggregation uses pairwise
AllGather of node features between the two halves.

LSTM layout: hidden/gate dim on partitions, nodes on the free dim, node
chunks ping-ponging over 2x4 PSUM banks.  Gate nonlinearities use ONE
sigmoid ACT op over the 4 strided slots [i|f|o|g]; the g slot holds 2*a_g
(host-side 2x weight scale) so sigmoid gives Sg=(tanh(a_g)+1)/2:
  PSUM = W_hh[g] @ h + W_ih[g] @ x_t + b[g]     (g slot scaled 2x)
  S  = sigmoid(PSUM)             one ACT op per chunk  [i|f|o|g]
  Tg = 2*Sg - 1                  (DVE tensor_scalar, 4x rate)
  t2 = Si * Tg                   (DVE tensor_tensor, 2x rate)
  t1 = Sf * c                    (DVE tensor_tensor)
  c' = t1 + t2                   (DVE tensor_tensor)
  Tc = tanh(c')                  (ACT)
  h' = So * Tc                   (DVE tensor_tensor)
All state fp16 (DVE 2x/4x perf modes need 2-byte packed SBUF operands).
Biases enter through a ones-row on the x-side matmul (K=9).
"""

import os as _os

import ml_dtypes
import numpy as np

BFNP = np.float16

import concourse.bass as bass
import concourse.tile as tile
from concourse import bacc, mybir
from concourse.bass_utils import run_bass_kernel_spmd
from concourse.masks import make_identity

F32 = mybir.dt.float32
BF16 = mybir.dt.float16  # fp16: same PE rate as bf16, 8x finer mantissa

B, T, N, E = 4, 96, 2000, 16000
F_DYN, F_STA, H = 8, 16, 128
N_CORES = 8
NPC = B * N // N_CORES      # 1000 nodes per core
CH = NPC // 2               # 500 node chunk
GSL = 512                   # psum gate slot stride (one bank)

# module-level knobs (test.py may override)
TRACE = False
TRACE_KW = {}

_PROG_CACHE = {}


def _build_program(t_steps=T, repeat=1, gnn_repeat=1):
    nc = bacc.Bacc("TRN2", target_bir_lowering=False, debug=False,
                   num_devices=N_CORES)

    assert t_steps % 3 == 0
    tb = t_steps // 3  # x blocks of 3 timesteps (partition groups 0/32/64)

    # ---- DRAM I/O ----
    x_dram = nc.dram_tensor("x_dyn", [tb * 27, NPC], BF16, kind="ExternalInput")
    w_rec = nc.dram_tensor("w_rec", [H, 4 * H], BF16, kind="ExternalInput")
    w_x = nc.dram_tensor("w_x", [128, 4 * H], BF16, kind="ExternalInput")
    w_sta = nc.dram_tensor("w_sta", [F_STA + 1, H], F32, kind="ExternalInput")
    sta_t = nc.dram_tensor("sta_t", [F_STA + 1, NPC], F32, kind="ExternalInput")
    w_fz = nc.dram_tensor("w_fz", [H, H], BF16, kind="ExternalInput")
    w_fs = nc.dram_tensor("w_fs", [H, H], BF16, kind="ExternalInput")
    b_fu = nc.dram_tensor("b_fu", [H, 1], F32, kind="ExternalInput")
    w_r1 = nc.dram_tensor("w_r1", [H, H], BF16, kind="ExternalInput")
    w_l1 = nc.dram_tensor("w_l1", [H, H], BF16, kind="ExternalInput")
    b_l1 = nc.dram_tensor("b_l1", [H, 1], F32, kind="ExternalInput")
    w_r2 = nc.dram_tensor("w_r2", [H, H], BF16, kind="ExternalInput")
    w_l2 = nc.dram_tensor("w_l2", [H, H], BF16, kind="ExternalInput")
    b_l2 = nc.dram_tensor("b_l2", [H, 1], F32, kind="ExternalInput")
    w_ou = nc.dram_tensor("w_ou", [H, 1], BF16, kind="ExternalInput")
    b_ou = nc.dram_tensor("b_ou", [1, 1], F32, kind="ExternalInput")
    a_mat = nc.dram_tensor("a_mat", [N, N], BF16, kind="ExternalInput")
    out_d = nc.dram_tensor("out", [1, N], F32, kind="ExternalOutput")

    AT = mybir.AluOpType
    AF = mybir.ActivationFunctionType
    n_kchunks = (N + 127) // 128  # 16 src chunks for aggregation

    with tile.TileContext(nc) as tc:
        with (
            tc.tile_pool(name="const", bufs=1) as cp,
            tc.tile_pool(name="xp", bufs=3) as xp,
            tc.tile_pool(name="wk", bufs=2) as wk,
        ):
            # ---- constants into SBUF ----
            def cload(dram, shape, tag, dt=F32):
                tl = cp.tile(shape, dt, tag=tag)
                nc.sync.dma_start(out=tl[:, :], in_=dram[:, :])
                return tl

            w_rec_t = cload(w_rec, [H, 4 * H], "w_rec", BF16)
            w_x_t = cload(w_x, [128, 4 * H], "w_x", BF16)
            w_sta_tt = cload(w_sta, [F_STA + 1, H], "w_sta")
            sta_tt = cload(sta_t, [F_STA + 1, NPC], "sta_t")
            w_fz_t = cload(w_fz, [H, H], "w_fz", BF16)
            w_fs_t = cload(w_fs, [H, H], "w_fs", BF16)
            b_fu_t = cload(b_fu, [H, 1], "b_fu")
            w_r1_t = cload(w_r1, [H, H], "w_r1", BF16)
            w_l1_t = cload(w_l1, [H, H], "w_l1", BF16)
            b_l1_t = cload(b_l1, [H, 1], "b_l1")
            w_r2_t = cload(w_r2, [H, H], "w_r2", BF16)
            w_l2_t = cload(w_l2, [H, H], "w_l2", BF16)
            b_l2_t = cload(b_l2, [H, 1], "b_l2")
            w_ou_t = cload(w_ou, [H, 1], "w_ou", BF16)
            b_ou_t = cload(b_ou, [1, 1], "b_ou")

            a_tiles = []
            KC = N // 16  # 125-row src chunks over the FULL graph adjacency
            for k in range(16):
                tl = cp.tile([KC, N], BF16, tag=f"a{k}")
                nc.sync.dma_start(out=tl[0:KC, :],
                                  in_=a_mat[KC * k:KC * k + KC, :])
                a_tiles.append(tl)

            ident = cp.tile([128, 128], BF16, tag="ident")
            make_identity(nc, ident[:, :])

            # ---- LSTM ----
            # persistent psum gate tiles: [i|f|o|g] gate slots at 512 strides
            pl_cm = tc.tile_pool(name="psl", bufs=2, space="PSUM")
            pl = pl_cm.__enter__()
            ps_ch = [pl.tile([128, 4 * GSL], F32, tag="gates", name="gates0"),
                     pl.tile([128, 4 * GSL], F32, tag="gates", name="gates1")]
            # zero the pad columns the gate ACT op reads ([500:512] of each slot)
            for ps in ps_ch:
                nc.vector.memset(ps[:, :], 0.0)

            rep_cm = tc.For_i(0, repeat, 1) if repeat > 1 else None
            if rep_cm is not None:
                rep_cm.__enter__()

            def load_xblock(b):
                tl = xp.tile([128, NPC], BF16, tag="xb")
                for g in range(3):
                    nc.sync.dma_start(
                        out=tl[32 * g:32 * g + 9, :],
                        in_=x_dram[27 * b + 9 * g:27 * b + 9 * g + 9, :])
                return tl

            xtiles = {0: load_xblock(0)}
            if tb > 1:
                xtiles[1] = load_xblock(1)

            s_prev = None
            h_prev = None
            n_chunks = int(_os.environ.get("K_CHUNKS", "2"))
            CH_N = NPC // n_chunks
            half = max(n_chunks // 2, 1)  # chunks per psum tile
            for t in range(t_steps):
                blk, grp = divmod(t, 3)
                if grp == 0 and blk + 2 < tb:
                    xtiles[blk + 2] = load_xblock(blk + 2)
                xt = xtiles[blk]

                s_new = wk.tile([128, NPC], BF16, tag="s")
                h_new = wk.tile([128, NPC], BF16, tag="h",
                                bufs=3 if repeat > 1 else 2)
                tc_t = wk.tile([128, NPC], BF16, tag="tc", bufs=1)
                tts = {}

                def emit_tail(c):
                    # Tc = tanh(c'); h' = So * Tc   (for chunk c)
                    sl = slice(CH_N * c, CH_N * c + CH_N)
                    so = tts[c][:, 2 * CH_N:3 * CH_N]
                    nc.scalar.activation(
                        out=tc_t[:, sl], in_=s_new[:, sl],
                        func=AF.Tanh, scale=1.0)
                    nc.vector.tensor_tensor(
                        out=h_new[:, sl], in0=so, in1=tc_t[:, sl],
                        op=AT.mult)

                for c in range(n_chunks):
                    pair, c2 = divmod(c, half)
                    ps = ps_ch[pair]
                    off = 256 * c2 if half == 2 else 0
                    sl = slice(CH_N * c, CH_N * c + CH_N)
                    tt = wk.tile([128, 4 * CH_N], BF16, tag=f"T{c}",
                                 name=f"tt{c}")
                    tts[c] = tt

                    for gi in range(4):
                        osl = slice(GSL * gi + off, GSL * gi + off + CH_N)
                        nc.tensor.matmul(
                            out=ps[:, osl],
                            lhsT=w_x_t[32 * grp:32 * grp + 9, H * gi:H * gi + H],
                            rhs=xt[32 * grp:32 * grp + 9, sl],
                            start=True, stop=(t == 0))
                    if t > 0:
                        for gi in range(4):
                            osl = slice(GSL * gi + off, GSL * gi + off + CH_N)
                            nc.tensor.matmul(
                                out=ps[:, osl],
                                lhsT=w_rec_t[:, H * gi:H * gi + H],
                                rhs=h_prev[:, sl],
                                start=False, stop=True)

                    # S = sigmoid(psum): strided over the 4 gate slots
                    nc.scalar.activation(
                        out=tt[:, :].rearrange("p (g b) -> p g b", g=4),
                        in_=ps[:, :].rearrange("p (g b) -> p g b", g=4)
                        [:, :, off:off + CH_N],
                        func=AF.Sigmoid, scale=1.0)

                    si = tt[:, 0:CH_N]
                    sf = tt[:, CH_N:2 * CH_N]
                    sg = tt[:, 3 * CH_N:4 * CH_N]
                    # Tg = 2*Sg - 1  (tensor_scalar, 4x)
                    tg = wk.tile([128, CH_N], BF16, tag=f"g{c}",
                                 name=f"tg{c}")
                    nc.vector.tensor_scalar(
                        out=tg[:, :], in0=sg, scalar1=2.0, scalar2=-1.0,
                        op0=AT.mult, op1=AT.add)
                    if t > 0:
                        t1 = wk.tile([128, CH_N], BF16, tag=f"q{c}",
                                     name=f"t1{c}")
                        t2 = wk.tile([128, CH_N], BF16, tag=f"p{c}",
                                     name=f"t2{c}")
                        # t2 = Si*Tg ; t1 = Sf*c ; c' = t1 + t2  (TT, 2x)
                        nc.vector.tensor_tensor(
                            out=t2[:, :], in0=si, in1=tg[:, :], op=AT.mult)
                        nc.vector.tensor_tensor(
                            out=t1[:, :], in0=sf, in1=s_prev[:, sl],
                            op=AT.mult)
                        nc.vector.tensor_tensor(
                            out=s_new[:, sl], in0=t1[:, :], in1=t2[:, :],
                            op=AT.add)
                    else:
                        # c0 = Si*Tg
                        nc.vector.tensor_tensor(
                            out=s_new[:, sl], in0=si, in1=tg[:, :],
                            op=AT.mult)

                    # software pipeline: previous chunk's tanh(c')/h after
                    # this chunk's gate ACT, so Tc's DVE wait doesn't block
                    # the next gate op in the in-order ACT queue
                    if c > 0:
                        emit_tail(c - 1)
                emit_tail(n_chunks - 1)

                s_prev, h_prev = s_new, h_new

            hh = h_prev  # [128, NPC] = h_final
            # repeat (device For_i) wraps ONLY the collective-free LSTM;
            # the GNN phase is python-unrolled via gnn_repeat instead
            # (collectives inside a hardware loop wedge the runtime).
            if rep_cm is not None:
                rep_cm.__exit__(None, None, None)
                rep_cm = None
            if _os.environ.get("K_SKIP_GNN"):
                pred0 = wk.tile([1, N], F32, tag="pred0", bufs=1)
                nc.vector.tensor_copy(out=pred0[0:1, 0:NPC], in_=hh[0:1, :])
                nc.vector.tensor_copy(out=pred0[0:1, NPC:N], in_=hh[0:1, :])
                nc.sync.dma_start(out=out_d[0:1, :], in_=pred0[0:1, :])
                pl_cm.__exit__(None, None, None)
                return nc
            pl_cm.__exit__(None, None, None)
            pp_cm = tc.tile_pool(name="psg", bufs=2, space="PSUM")
            pp = pp_cm.__enter__()

            # ---- static encoder + fusion (own NPC nodes) ----
            def mm_halves(psum, pairs, width=NPC):
                # pairs: (lhsT_ap, rhs_tile, rhs_partitions); 500-col slots
                # at GSL strides (psum accumulation groups stay in-bank)
                nh = width // CH
                for c in range(nh):
                    osl = slice(GSL * c, GSL * c + CH)
                    for j, (lt, rtile, pr) in enumerate(pairs):
                        nc.tensor.matmul(
                            out=psum[:, osl], lhsT=lt,
                            rhs=rtile[0:pr, CH * c:CH * c + CH],
                            start=(j == 0), stop=(j == len(pairs) - 1))

            def psum_drain(psum, dst, func, bias=0.0, width=NPC, rows=128):
                # one strided ACT op over the GSL-strided slots
                nh = width // CH
                nc.scalar.activation(
                    out=dst[0:rows, 0:width].rearrange("p (c b) -> p c b", c=nh),
                    in_=psum[0:rows, 0:GSL * nh].rearrange(
                        "p (c b) -> p c b", c=nh)[:, :, 0:CH],
                    func=func, bias=bias, scale=1.0)

            stl = wk.tile([128, NPC], BF16, tag="stl", bufs=1)
            pss = pp.tile([128, 2 * GSL], F32, tag="gp")
            mm_halves(pss, [(w_sta_tt[0:17, :], sta_tt, 17)])
            psum_drain(pss, stl, AF.Relu)

            node_t = wk.tile([128, NPC], BF16, tag="node", bufs=1)
            psf = pp.tile([128, 2 * GSL], F32, tag="gp")
            mm_halves(psf, [(w_fz_t[:, :], hh, 128), (w_fs_t[:, :], stl, 128)])
            psum_drain(psf, node_t, AF.Relu, bias=b_fu_t[:, 0:1])

            # ---- single pairwise exchange of fused node features ----
            with tc.tile_pool(name="dram", bufs=1, space="DRAM") as dp:
                cc_in = dp.tile([128, NPC], BF16, tag="ci")
                cc_out = dp.tile([256, NPC], BF16, tag="co")
                nc.sync.dma_start(out=cc_in[:, :], in_=node_t[:, :])
                nc.gpsimd.collective_compute(
                    "AllGather", AT.bypass,
                    replica_groups=[[0, 1], [2, 3], [4, 5], [6, 7]],
                    ins=[cc_in.opt()], outs=[cc_out.opt()])

                # z_all: fused features of the FULL graph, feature-major
                z_all = wk.tile([128, N], BF16, tag="zall", bufs=1)
                for r in range(2):
                    nc.sync.dma_start(
                        out=z_all[:, NPC * r:NPC * r + NPC],
                        in_=cc_out[128 * r:128 * r + 128, :])

                def to_node_major(x_t_tile, tag):
                    # [128, N] feature-major -> 16 chunks [KC, 128] node-major
                    x_nm = wk.tile([128, 16 * 128], BF16, tag=tag, bufs=1)
                    for k in range(16):
                        trp = pp.tile([128, 128], BF16, tag="tr")
                        nc.tensor.transpose(
                            out=trp[0:KC, :],
                            in_=x_t_tile[:, KC * k:KC * k + KC],
                            identity=ident[:, :])
                        nc.vector.tensor_copy(
                            out=x_nm[0:KC, 128 * k:128 * k + 128],
                            in_=trp[0:KC, :])
                    return x_nm

                def sage_full(x_t_tile, x_nm, w_r, w_l, b_l, relu, lname):
                    # mean/transform for ALL N dst nodes, in halves of NPC
                    out_t = wk.tile([128, N], BF16, tag=f"o{lname}", bufs=1)
                    mean_t = wk.tile([128, N], BF16, tag=f"m{lname}", bufs=1)
                    for hf in range(2):
                        dsl = slice(NPC * hf, NPC * hf + NPC)
                        psm = pp.tile([128, 2 * GSL], F32, tag="gp")
                        for k in range(16):
                            for dc in range(2):
                                osl = slice(GSL * dc, GSL * dc + CH)
                                nc.tensor.matmul(
                                    out=psm[:, osl],
                                    lhsT=x_nm[0:KC, 128 * k:128 * k + 128],
                                    rhs=a_tiles[k][0:KC,
                                                   NPC * hf + CH * dc:
                                                   NPC * hf + CH * dc + CH],
                                    start=(k == 0), stop=(k == 15))
                        psum_drain(psm, mean_t[:, dsl], AF.Identity)
                        psh = pp.tile([128, 2 * GSL], F32, tag="gp")
                        mm_halves(psh, [(w_r[:, :], x_t_tile[:, dsl], 128),
                                        (w_l[:, :], mean_t[:, dsl], 128)])
                        psum_drain(psh, out_t[:, dsl],
                                   AF.Relu if relu else AF.Identity,
                                   bias=b_l[:, 0:1])
                    return out_t

                for _gr in range(gnn_repeat):
                    z_nm = to_node_major(z_all, "znm")
                    h1_t = sage_full(z_all, z_nm, w_r1_t, w_l1_t, b_l1_t,
                                     True, "1")
                    h1_nm = to_node_major(h1_t, "hnm")
                    h2_t = sage_full(h1_t, h1_nm, w_r2_t, w_l2_t, b_l2_t,
                                     False, "2")

                # ---- head (full graph, in halves) ----
                pred = wk.tile([1, N], F32, tag="pred", bufs=1)
                for hf in range(2):
                    pso = pp.tile([1, 2 * GSL], F32, tag="ho", bufs=1)
                    for dc in range(2):
                        nc.tensor.matmul(
                            out=pso[0:1, GSL * dc:GSL * dc + CH],
                            lhsT=w_ou_t[:, 0:1],
                            rhs=h2_t[:, NPC * hf + CH * dc:
                                     NPC * hf + CH * dc + CH],
                            start=True, stop=True)
                    nc.vector.tensor_scalar(
                        out=pred[0:1, NPC * hf:NPC * hf + NPC].rearrange(
                            "p (c b) -> p c b", c=2),
                        in0=pso[0:1, :].rearrange("p (c b) -> p c b", c=2)
                        [:, :, 0:CH],
                        scalar1=b_ou_t[0:1, 0:1], scalar2=None, op0=AT.add)
                nc.sync.dma_start(out=out_d[0:1, :], in_=pred[0:1, :])
            pp_cm.__exit__(None, None, None)
            if rep_cm is not None:
                rep_cm.__exit__(None, None, None)

    return nc


def _prep_inputs(inputs, t_steps=T):
    """Host-side preprocessing: per-core input maps."""
    dyn = np.asarray(inputs["dynamic_features"], np.float32)
    sta = np.asarray(inputs["static_features"], np.float32)
    ei = np.asarray(inputs["edge_index"])
    W_ih = np.asarray(inputs["W_ih"], np.float32)
    W_hh = np.asarray(inputs["W_hh"], np.float32)
    b = (np.asarray(inputs["b_ih"], np.float32)
         + np.asarray(inputs["b_hh"], np.float32))
    W_sta = np.asarray(inputs["W_sta"], np.float32)
    b_sta = np.asarray(inputs["b_sta"], np.float32)
    W_fuse = np.asarray(inputs["W_fuse"], np.float32)
    b_fuse = np.asarray(inputs["b_fuse"], np.float32)
    s1_Wl = np.asarray(inputs["sage1_Wl"], np.float32)
    s1_bl = np.asarray(inputs["sage1_bl"], np.float32)
    s1_Wr = np.asarray(inputs["sage1_Wr"], np.float32)
    s2_Wl = np.asarray(inputs["sage2_Wl"], np.float32)
    s2_bl = np.asarray(inputs["sage2_bl"], np.float32)
    s2_Wr = np.asarray(inputs["sage2_Wr"], np.float32)
    W_out = np.asarray(inputs["W_out"], np.float32)
    b_out = np.asarray(inputs["b_out"], np.float32)

    tb = t_steps // 3

    # gate order in psum: [i, f, o, g]; torch order in weights: i,f,g,o
    gsl = [slice(0, H), slice(H, 2 * H), slice(3 * H, 4 * H), slice(2 * H, 3 * H)]
    # w_rec: lhsT [h_in, 4H]; g-gate 2x (tanh via sigmoid of doubled preact)
    w_rec = np.concatenate(
        [W_hh[gsl[0]].T, W_hh[gsl[1]].T,
         W_hh[gsl[2]].T, 2.0 * W_hh[gsl[3]].T], axis=1).astype(BFNP)
    # w_x: [9, 4H] = [W_ih^T; bias row], g-gate 2x; replicated at 4x32 partitions
    wx9 = np.zeros((9, 4 * H), np.float32)
    for j, s in enumerate(gsl):
        sc = 2.0 if j == 3 else 1.0
        wx9[0:8, H * j:H * j + H] = sc * W_ih[s].T
        wx9[8, H * j:H * j + H] = sc * b[s]
    w_x = np.zeros((128, 4 * H), BFNP)
    for g in range(3):
        w_x[32 * g:32 * g + 9] = wx9.astype(BFNP)

    w_sta_t = np.zeros((F_STA + 1, H), np.float32)
    w_sta_t[0:F_STA] = W_sta.T
    w_sta_t[F_STA] = b_sta

    w_fz = np.ascontiguousarray(W_fuse[:, :H].T.astype(BFNP))
    w_fs = np.ascontiguousarray(W_fuse[:, H:].T)

    # normalized adjacency (same graph for every batch element)
    src, dst = ei[0].astype(np.int64), ei[1].astype(np.int64)
    cnt = np.bincount(dst, minlength=N).astype(np.float32)
    A = np.zeros((N, N), np.float32)
    np.add.at(A, (src, dst), 1.0)
    A /= np.maximum(cnt, 1.0)[None, :]
    A16 = np.ascontiguousarray(A).astype(BFNP)

    x_bn = dyn.transpose(0, 2, 1, 3).reshape(B * N, dyn.shape[1], F_DYN)
    sta_bn = sta.reshape(B * N, F_STA)

    shared = dict(
        w_rec=w_rec, w_x=w_x, w_sta=w_sta_t,
        w_fz=w_fz, w_fs=np.ascontiguousarray(W_fuse[:, H:].T.astype(BFNP)),
        b_fu=b_fuse.reshape(H, 1),
        w_r1=np.ascontiguousarray(s1_Wr.T.astype(BFNP)),
        w_l1=np.ascontiguousarray(s1_Wl.T.astype(BFNP)),
        b_l1=s1_bl.reshape(H, 1),
        w_r2=np.ascontiguousarray(s2_Wr.T.astype(BFNP)),
        w_l2=np.ascontiguousarray(s2_Wl.T.astype(BFNP)),
        b_l2=s2_bl.reshape(H, 1),
        w_ou=np.ascontiguousarray(W_out.T.astype(BFNP)),
        b_ou=b_out.reshape(1, 1),
    )

    in_maps = []
    for core in range(N_CORES):
        rows = slice(NPC * core, NPC * core + NPC)
        xc = x_bn[rows, 0:t_steps, :]                       # [NPC, T, 8]
        xt = xc.transpose(1, 2, 0)                          # [T, 8, NPC]
        arr = np.ones((tb, 3, 9, NPC), np.float32)
        arr[:, :, 0:8, :] = xt.reshape(tb, 3, 8, NPC)
        x_in = np.ascontiguousarray(arr.reshape(tb * 27, NPC)).astype(BFNP)

        sta_in = np.ones((F_STA + 1, NPC), np.float32)
        sta_in[0:F_STA] = sta_bn[rows].T

        m = dict(shared)
        m.update(x_dyn=x_in, sta_t=sta_in, a_mat=A16)
        in_maps.append(m)
    return in_maps


def kernel(**inputs):
    t_steps = int(np.asarray(inputs["dynamic_features"]).shape[1])
    if t_steps not in _PROG_CACHE:
        nc_new = _build_program(t_steps)
        if not nc_new.is_finalized():
            nc_new.finalize()
        _PROG_CACHE[t_steps] = nc_new
    nc = _PROG_CACHE[t_steps]
    in_maps = _prep_inputs(inputs, t_steps)
    br = run_bass_kernel_spmd(nc, in_maps, list(range(N_CORES)),
                              trace=TRACE, **TRACE_KW)
    kernel.last_result = br
    # each pair core holds the full graph prediction; take the even cores'
    out = np.stack(
        [np.asarray(br.results[2 * g]["out"]).reshape(N) for g in range(B)])
    return out.astype(np.float32)



# revision 64
# speedup vs baseline: 74.2754x; 74.2754x over previous
"""Trainium2 Bass kernel for CombinedLSTMWithStatic2Hop.

Model: per-node LSTM over T timesteps + static encoder -> fusion -> 2x SAGEConv
(mean aggregation) -> linear head.

Sharding: B*N = 8000 nodes split into 1000 contiguous nodes per core (8 cores).
Each graph (2000 nodes) spans a core pair; SAGE aggregation uses pairwise
AllGather of node features between the two halves.

LSTM layout: hidden/gate dim on partitions, nodes on the free dim, node
chunks ping-ponging over 2x4 PSUM banks.  Gate nonlinearities use ONE
sigmoid ACT op over the 4 strided slots [i|f|o|g]; the g slot holds 2*a_g
(host-side 2x weight scale) so sigmoid gives Sg=(tanh(a_g)+1)/2:
  PSUM = W_hh[g] @ h + W_ih[g] @ x_t + b[g]     (g slot scaled 2x)
  S  = sigmoid(PSUM)             one ACT op per chunk  [i|f|o|g]
  Tg = 2*Sg - 1                  (DVE tensor_scalar, 4x rate)
  t2 = Si * Tg                   (DVE tensor_tensor, 2x rate)
  t1 = Sf * c                    (DVE tensor_tensor)
  c' = t1 + t2                   (DVE tensor_tensor)
  Tc = tanh(c')                  (ACT)
  h' = So * Tc                   (DVE tensor_tensor)
All state fp16 (DVE 2x/4x perf modes need 2-byte packed SBUF operands).
Biases enter through a ones-row on the x-side matmul (K=9).
"""

import os as _os

import ml_dtypes
import numpy as np

BFNP = np.float16

import concourse.bass as bass
import concourse.tile as tile
from concourse import bacc, mybir
from concourse.bass_utils import run_bass_kernel_spmd
from concourse.masks import make_identity

F32 = mybir.dt.float32
BF16 = mybir.dt.float16  # fp16: same PE rate as bf16, 8x finer mantissa
E4 = mybir.dt.float8e4   # fp8 e4m3: DoubleRow matmul at 2 cols/cycle
E4NP = ml_dtypes.float8_e4m3fn
FP8 = _os.environ.get("K_FP8", "1") != "0"

B, T, N, E = 4, 96, 2000, 16000
F_DYN, F_STA, H = 8, 16, 128
N_CORES = 8
NPC = B * N // N_CORES      # 1000 nodes per core
CH = NPC // 2               # 500 node chunk
GSL = 512                   # psum gate slot stride (one bank)

# module-level knobs (test.py may override)
TRACE = False
TRACE_KW = {}

_PROG_CACHE = {}


def _build_program(t_steps=T, repeat=1, gnn_repeat=1):
    nc = bacc.Bacc("TRN2", target_bir_lowering=False, debug=False,
                   num_devices=N_CORES)

    assert t_steps % 3 == 0
    tb = t_steps // 3  # x blocks of 3 timesteps (partition groups 0/32/64)

    # ---- DRAM I/O ----
    x_dram = nc.dram_tensor("x_dyn", [tb * 27, NPC], BF16, kind="ExternalInput")
    w_rec = nc.dram_tensor("w_rec", [H, 4 * H], BF16, kind="ExternalInput")
    w_x = nc.dram_tensor("w_x", [128, 4 * H], BF16, kind="ExternalInput")
    # fp8 fused path: per-t full-partition x images (zeros outside the
    # active 9-row group) + [W_hh | W_x-group] DoubleRow weights
    x8_dram = nc.dram_tensor("x8", [t_steps * 128, NPC], E4,
                             kind="ExternalInput")
    w_f = nc.dram_tensor("w_f", [128, 3 * 2 * 4 * H], E4,
                         kind="ExternalInput")
    w_sta = nc.dram_tensor("w_sta", [F_STA + 1, H], F32, kind="ExternalInput")
    sta_t = nc.dram_tensor("sta_t", [F_STA + 1, NPC], F32, kind="ExternalInput")
    w_fz = nc.dram_tensor("w_fz", [H, H], BF16, kind="ExternalInput")
    w_fs = nc.dram_tensor("w_fs", [H, H], BF16, kind="ExternalInput")
    b_fu = nc.dram_tensor("b_fu", [H, 1], F32, kind="ExternalInput")
    w_r1 = nc.dram_tensor("w_r1", [H, H], BF16, kind="ExternalInput")
    w_l1 = nc.dram_tensor("w_l1", [H, H], BF16, kind="ExternalInput")
    b_l1 = nc.dram_tensor("b_l1", [H, 1], F32, kind="ExternalInput")
    w_r2 = nc.dram_tensor("w_r2", [H, H], BF16, kind="ExternalInput")
    w_l2 = nc.dram_tensor("w_l2", [H, H], BF16, kind="ExternalInput")
    b_l2 = nc.dram_tensor("b_l2", [H, 1], F32, kind="ExternalInput")
    w_ou = nc.dram_tensor("w_ou", [H, 1], BF16, kind="ExternalInput")
    b_ou = nc.dram_tensor("b_ou", [1, 1], F32, kind="ExternalInput")
    a_mat = nc.dram_tensor("a_mat", [N, N], BF16, kind="ExternalInput")
    out_d = nc.dram_tensor("out", [1, N], F32, kind="ExternalOutput")

    AT = mybir.AluOpType
    AF = mybir.ActivationFunctionType
    n_kchunks = (N + 127) // 128  # 16 src chunks for aggregation

    with tile.TileContext(nc) as tc:
        with (
            tc.tile_pool(name="const", bufs=1) as cp,
            tc.tile_pool(name="xp", bufs=3) as xp,
            tc.tile_pool(name="wk", bufs=2) as wk,
        ):
            # ---- constants into SBUF ----
            def cload(dram, shape, tag, dt=F32):
                tl = cp.tile(shape, dt, tag=tag)
                nc.sync.dma_start(out=tl[:, :], in_=dram[:, :])
                return tl

            if FP8:
                w_f_t = cload(w_f, [128, 3 * 2 * 4 * H], "w_f", E4)
            else:
                w_rec_t = cload(w_rec, [H, 4 * H], "w_rec", BF16)
                w_x_t = cload(w_x, [128, 4 * H], "w_x", BF16)
            w_sta_tt = cload(w_sta, [F_STA + 1, H], "w_sta")
            sta_tt = cload(sta_t, [F_STA + 1, NPC], "sta_t")
            w_fz_t = cload(w_fz, [H, H], "w_fz", BF16)
            w_fs_t = cload(w_fs, [H, H], "w_fs", BF16)
            b_fu_t = cload(b_fu, [H, 1], "b_fu")
            w_r1_t = cload(w_r1, [H, H], "w_r1", BF16)
            w_l1_t = cload(w_l1, [H, H], "w_l1", BF16)
            b_l1_t = cload(b_l1, [H, 1], "b_l1")
            w_r2_t = cload(w_r2, [H, H], "w_r2", BF16)
            w_l2_t = cload(w_l2, [H, H], "w_l2", BF16)
            b_l2_t = cload(b_l2, [H, 1], "b_l2")
            w_ou_t = cload(w_ou, [H, 1], "w_ou", BF16)
            b_ou_t = cload(b_ou, [1, 1], "b_ou")

            ident = cp.tile([128, 128], BF16, tag="ident")
            make_identity(nc, ident[:, :])

            # 8MB of adjacency on the (idle) gpsimd DMA queue so it streams
            # during the LSTM instead of delaying the x loads / t=0
            a_tiles = []
            KC = N // 16  # 125-row src chunks over the FULL graph adjacency
            for k in range(16):
                tl = cp.tile([KC, N], BF16, tag=f"a{k}")
                nc.gpsimd.dma_start(out=tl[0:KC, :],
                                    in_=a_mat[KC * k:KC * k + KC, :])
                a_tiles.append(tl)

            # ---- LSTM ----
            # persistent psum gate tiles: [i|f|o|g] gate slots at 512 strides
            pl_cm = tc.tile_pool(name="psl", bufs=2, space="PSUM")
            pl = pl_cm.__enter__()

            # static encoder first: fills the ACT startup gap while the
            # first x DMAs land; reuses a gate psum buffer before the LSTM
            stl = wk.tile([128, NPC], BF16, tag="stl", bufs=1)
            pss = pl.tile([128, 4 * GSL], F32, tag="gates", name="pss")
            for dc in range(2):
                nc.tensor.matmul(
                    out=pss[:, GSL * dc:GSL * dc + CH],
                    lhsT=w_sta_tt[0:17, :],
                    rhs=sta_tt[0:17, CH * dc:CH * dc + CH],
                    start=True, stop=True)
            nc.scalar.activation(
                out=stl[:, :].rearrange("p (c b) -> p c b", c=2),
                in_=pss[:, 0:2 * GSL].rearrange("p (c b) -> p c b", c=2)
                [:, :, 0:CH],
                func=AF.Relu, scale=1.0)

            ps_ch = [pl.tile([128, 4 * GSL], F32, tag="gates", name="gates0"),
                     pl.tile([128, 4 * GSL], F32, tag="gates", name="gates1")]
            # zero the pad columns the gate ACT op reads ([500:512] of each slot)
            for ps in ps_ch:
                nc.vector.memset(ps[:, :], 0.0)

            rep_cm = tc.For_i(0, repeat, 1) if repeat > 1 else None
            if rep_cm is not None:
                rep_cm.__enter__()

            def load_xblock(b):
                tl = xp.tile([128, NPC], BF16, tag="xb")
                for g in range(3):
                    nc.sync.dma_start(
                        out=tl[32 * g:32 * g + 9, :],
                        in_=x_dram[27 * b + 9 * g:27 * b + 9 * g + 9, :])
                return tl

            CH_N = NPC // 4          # matmul chunk (250): psum write size
            PR = int(_os.environ.get("K_PRW", str(NPC // 4)))  # ACT/DVE width
            n_groups = NPC // PR
            mm_per_g = PR // CH_N
            TCW = int(_os.environ.get("K_TCW", str(NPC // 2)))  # Tc width
            tgr = TCW // PR          # groups per tail block
            h_bufs = (4 if repeat > 1 else 3) if FP8 else \
                     (3 if repeat > 1 else 2)

            if FP8:
                # combined [h | x] tiles: left half h(t), right half x(t+1).
                # Weight rows outside the active 9-row x group are zero, so
                # stale x data in other groups' rows cannot contaminate;
                # memset once so no row is ever read uninitialized.
                _xh_n = [0]

                def xh_alloc():
                    _xh_n[0] += 1
                    return wk.tile([128, 2 * NPC], E4, tag="h", bufs=h_bufs,
                                   name=f"xh{_xh_n[0]}")

                def dma_x8(tile, t):
                    nc.sync.dma_start(
                        out=tile[:, NPC:2 * NPC],
                        in_=x8_dram[128 * t:128 * t + 128, :])

                xh_prev = xh_alloc()        # XH[-1]: h(-1)=0 | x(0)
                nc.vector.memset(xh_prev[:, 0:NPC], 0.0)
                dma_x8(xh_prev, 0)
                w_f_r = w_f_t[:, :].rearrange("p (x s m) -> p x s m",
                                              x=3, s=2)
            else:
                xtiles = {0: load_xblock(0)}
                if tb > 1:
                    xtiles[1] = load_xblock(1)

            s_prev = None
            h_prev = None
            for t in range(t_steps):
                blk, grp = divmod(t, 3)
                if not FP8:
                    if grp == 0 and blk + 2 < tb:
                        xtiles[blk + 2] = load_xblock(blk + 2)
                    xt = xtiles[blk]

                s_new = wk.tile([128, NPC], BF16, tag="s")
                if FP8:
                    xh_cur = xh_alloc()
                    if t + 1 < t_steps:
                        dma_x8(xh_cur, t + 1)
                    h_new = xh_cur
                    xh_rhs = xh_prev[:, :].rearrange("p (s n) -> p s n", s=2)
                else:
                    h_new = wk.tile([128, NPC], BF16, tag="h", bufs=h_bufs)
                tc_t = wk.tile([128, NPC], BF16, tag="tc", bufs=1)
                tts = {}
                tails = 0

                def emit_tail(b):
                    # Tc = tanh(c') for tail-block b (TCW cols), then
                    # h' = So * Tc per group (So lives in per-group tiles)
                    bsl = slice(TCW * b, TCW * b + TCW)
                    nc.scalar.activation(
                        out=tc_t[:, bsl], in_=s_new[:, bsl],
                        func=AF.Tanh, scale=1.0)
                    for pr in range(b * tgr, (b + 1) * tgr):
                        psl = slice(PR * pr, PR * pr + PR)
                        so = tts[pr][:, 2 * PR:3 * PR]
                        nc.vector.tensor_tensor(
                            out=h_new[:, psl], in0=so, in1=tc_t[:, psl],
                            op=AT.mult)

                for pr in range(n_groups):
                    base = PR * pr
                    ps = ps_ch[base // (NPC // 2)]
                    in_off = base % (NPC // 2)
                    psl = slice(base, base + PR)
                    tt = wk.tile([128, 4 * PR], BF16, tag=f"T{pr}",
                                 name=f"tt{pr}")
                    tts[pr] = tt

                    # matmuls in 250-col chunks; a psum tile's groups are
                    # CONTIGUOUS in each 512-col gate slot (off 0/250) so
                    # one strided ACT op covers a whole group
                    for c2 in range(mm_per_g):
                        sl = slice(base + CH_N * c2, base + CH_N * c2 + CH_N)
                        po = in_off + CH_N * c2
                        for gi in range(4):
                            osl = slice(GSL * gi + po, GSL * gi + po + CH_N)
                            if FP8:
                                # one DoubleRow mm: W_hh@h + W_xg@x, 2 col/cyc
                                nc.tensor.matmul(
                                    out=ps[:, osl],
                                    lhsT=w_f_r[:, grp, :,
                                               H * gi:H * gi + H],
                                    rhs=xh_rhs[:, :, sl],
                                    start=True, stop=True,
                                    perf_mode=mybir.MatmulPerfMode.DoubleRow)
                            else:
                                nc.tensor.matmul(
                                    out=ps[:, osl],
                                    lhsT=w_x_t[32 * grp:32 * grp + 9,
                                               H * gi:H * gi + H],
                                    rhs=xt[32 * grp:32 * grp + 9, sl],
                                    start=True, stop=(t == 0))
                        if t > 0 and not FP8:
                            for gi in range(4):
                                osl = slice(GSL * gi + po,
                                            GSL * gi + po + CH_N)
                                nc.tensor.matmul(
                                    out=ps[:, osl],
                                    lhsT=w_rec_t[:, H * gi:H * gi + H],
                                    rhs=h_prev[:, sl],
                                    start=False, stop=True)

                    # S = sigmoid(psum): one op, 4 strided slots x PR
                    nc.scalar.activation(
                        out=tt[:, :].rearrange("p (g b) -> p g b", g=4),
                        in_=ps[:, :].rearrange("p (g b) -> p g b", g=4)
                        [:, :, in_off:in_off + PR],
                        func=AF.Sigmoid, scale=1.0)

                    si = tt[:, 0:PR]
                    sf = tt[:, PR:2 * PR]
                    sg = tt[:, 3 * PR:4 * PR]
                    # Tg = 2*Sg - 1  (tensor_scalar, 4x)
                    tg = wk.tile([128, PR], BF16, tag=f"g{pr}",
                                 name=f"tg{pr}")
                    nc.vector.tensor_scalar(
                        out=tg[:, :], in0=sg, scalar1=2.0, scalar2=-1.0,
                        op0=AT.mult, op1=AT.add)
                    if t > 0:
                        t1 = wk.tile([128, PR], BF16, tag=f"q{pr}",
                                     name=f"t1{pr}")
                        t2 = wk.tile([128, PR], BF16, tag=f"p{pr}",
                                     name=f"t2{pr}")
                        # t2 = Si*Tg ; t1 = Sf*c ; c' = t1 + t2  (TT, 2x)
                        t1_eng = (nc.gpsimd if _os.environ.get("K_T1_POOL")
                                  else nc.vector)
                        t1_eng.tensor_tensor(
                            out=t1[:, :], in0=sf, in1=s_prev[:, psl],
                            op=AT.mult)
                        nc.vector.tensor_tensor(
                            out=t2[:, :], in0=si, in1=tg[:, :], op=AT.mult)
                        nc.vector.tensor_tensor(
                            out=s_new[:, psl], in0=t1[:, :], in1=t2[:, :],
                            op=AT.add)
                    else:
                        # c0 = Si*Tg
                        nc.vector.tensor_tensor(
                            out=s_new[:, psl], in0=si, in1=tg[:, :],
                            op=AT.mult)

                    # software pipeline: a tail block's tanh(c')/h goes
                    # after the NEXT group's gate ACT, so Tc's DVE wait
                    # doesn't block gate ops in the in-order ACT queue
                    while (tails + 1) * tgr <= pr:
                        emit_tail(tails)
                        tails += 1
                while tails * tgr < n_groups:
                    emit_tail(tails)
                    tails += 1

                s_prev, h_prev = s_new, h_new
                if FP8:
                    xh_prev = xh_cur

            if FP8:
                # cast the final h (fp8) to fp16 for the fusion matmul
                hh = wk.tile([128, NPC], BF16, tag="hh16", bufs=1)
                nc.vector.tensor_copy(out=hh[:, :], in_=h_prev[:, 0:NPC])
            else:
                hh = h_prev  # [128, NPC] = h_final
            # repeat (device For_i) wraps ONLY the collective-free LSTM;
            # the GNN phase is python-unrolled via gnn_repeat instead
            # (collectives inside a hardware loop wedge the runtime).
            if rep_cm is not None:
                rep_cm.__exit__(None, None, None)
                rep_cm = None
            if _os.environ.get("K_SKIP_GNN"):
                pred0 = wk.tile([1, N], F32, tag="pred0", bufs=1)
                nc.vector.tensor_copy(out=pred0[0:1, 0:NPC], in_=hh[0:1, :])
                nc.vector.tensor_copy(out=pred0[0:1, NPC:N], in_=hh[0:1, :])
                nc.sync.dma_start(out=out_d[0:1, :], in_=pred0[0:1, :])
                pl_cm.__exit__(None, None, None)
                return nc
            pl_cm.__exit__(None, None, None)
            pp_cm = tc.tile_pool(name="psg", bufs=2, space="PSUM")
            pp = pp_cm.__enter__()

            # ---- static encoder + fusion (own NPC nodes) ----
            def mm_halves(psum, pairs, width=NPC):
                # pairs: (lhsT_ap, rhs_tile, rhs_partitions); 500-col slots
                # at GSL strides (psum accumulation groups stay in-bank)
                nh = width // CH
                for c in range(nh):
                    osl = slice(GSL * c, GSL * c + CH)
                    for j, (lt, rtile, pr) in enumerate(pairs):
                        nc.tensor.matmul(
                            out=psum[:, osl], lhsT=lt,
                            rhs=rtile[0:pr, CH * c:CH * c + CH],
                            start=(j == 0), stop=(j == len(pairs) - 1))

            def psum_drain(psum, dst, func, bias=0.0, width=NPC, rows=128):
                # one strided ACT op over the GSL-strided slots
                nh = width // CH
                nc.scalar.activation(
                    out=dst[0:rows, 0:width].rearrange("p (c b) -> p c b", c=nh),
                    in_=psum[0:rows, 0:GSL * nh].rearrange(
                        "p (c b) -> p c b", c=nh)[:, :, 0:CH],
                    func=func, bias=bias, scale=1.0)

            node_t = wk.tile([128, NPC], BF16, tag="node", bufs=1)
            psf = pp.tile([128, 2 * GSL], F32, tag="gp")
            mm_halves(psf, [(w_fz_t[:, :], hh, 128), (w_fs_t[:, :], stl, 128)])
            psum_drain(psf, node_t, AF.Relu, bias=b_fu_t[:, 0:1])

            # ---- single pairwise exchange of fused node features ----
            # All aggregation/dst ordering is [own | peer] per core (host
            # permutes odd cores' adjacency to match); even cores' order is
            # global and only their outputs are read back.
            with tc.tile_pool(name="dram", bufs=1, space="DRAM") as dp:
                cc_in = dp.tile([128, NPC], BF16, tag="ci")
                cc_out = dp.tile([256, NPC], BF16, tag="co")
                nc.sync.dma_start(out=cc_in[:, :], in_=node_t[:, :])
                nc.gpsimd.collective_compute(
                    "AllGather", AT.bypass,
                    replica_groups=[[0, 1], [2, 3], [4, 5], [6, 7]],
                    ins=[cc_in.opt()], outs=[cc_out.opt()])

                def trans_chunks(x_t_tile, x_nm, k0):
                    # [128, NPC] feature-major -> 8 chunks [KC, 128] at k0..
                    for k in range(8):
                        trp = pp.tile([128, 128], BF16, tag="tr")
                        nc.tensor.transpose(
                            out=trp[0:KC, :],
                            in_=x_t_tile[:, KC * k:KC * k + KC],
                            identity=ident[:, :])
                        nc.vector.tensor_copy(
                            out=x_nm[0:KC, 128 * (k0 + k):128 * (k0 + k) + 128],
                            in_=trp[0:KC, :])

                def agg_emit(x_nm, psms, ks):
                    for k in ks:
                        for hf in range(2):
                            for dc in range(2):
                                osl = slice(GSL * dc, GSL * dc + CH)
                                nc.tensor.matmul(
                                    out=psms[hf][:, osl],
                                    lhsT=x_nm[0:KC, 128 * k:128 * k + 128],
                                    rhs=a_tiles[k][0:KC,
                                                   NPC * hf + CH * dc:
                                                   NPC * hf + CH * dc + CH],
                                    start=(k == 0), stop=(k == 15))

                for _gr in range(gnn_repeat):
                    # own-half transposes + own-src partial aggregation run
                    # UNDER the collective (they only need local node_t)
                    z_nm = wk.tile([128, 16 * 128], BF16, tag="znm", bufs=1)
                    trans_chunks(node_t, z_nm, 0)
                    psm0 = pp.tile([128, 2 * GSL], F32, tag="gp")
                    psm1 = pp.tile([128, 2 * GSL], F32, tag="gp")
                    agg_emit(z_nm, [psm0, psm1], range(0, 8))

                    # peer features: (block0 + block1) - node_t, symmetric
                    zb = wk.tile([128, 2 * NPC], BF16, tag="zb", bufs=1)
                    for r in range(2):
                        nc.sync.dma_start(
                            out=zb[:, NPC * r:NPC * r + NPC],
                            in_=cc_out[128 * r:128 * r + 128, :])
                    z_peer = wk.tile([128, NPC], BF16, tag="zp", bufs=1)
                    nc.vector.tensor_tensor(
                        out=z_peer[:, :], in0=zb[:, 0:NPC],
                        in1=zb[:, NPC:2 * NPC], op=AT.add)
                    nc.vector.tensor_tensor(
                        out=z_peer[:, :], in0=z_peer[:, :],
                        in1=node_t[:, :], op=AT.subtract)
                    trans_chunks(z_peer, z_nm, 8)
                    agg_emit(z_nm, [psm0, psm1], range(8, 16))

                    mean_t = wk.tile([128, N], BF16, tag="m1", bufs=1)
                    h1_t = wk.tile([128, N], BF16, tag="o1", bufs=1)
                    roots = [node_t, z_peer]
                    for hf in range(2):
                        dsl = slice(NPC * hf, NPC * hf + NPC)
                        psum_drain([psm0, psm1][hf], mean_t[:, dsl],
                                   AF.Identity)
                        psh = pp.tile([128, 2 * GSL], F32, tag="gp")
                        mm_halves(psh, [(w_r1_t[:, :], roots[hf], 128),
                                        (w_l1_t[:, :], mean_t[:, dsl], 128)])
                        psum_drain(psh, h1_t[:, dsl], AF.Relu,
                                   bias=b_l1_t[:, 0:1])

                    # ---- SAGE 2 (all-local) ----
                    h1_nm = wk.tile([128, 16 * 128], BF16, tag="hnm", bufs=1)
                    trans_chunks(h1_t[:, 0:NPC], h1_nm, 0)
                    trans_chunks(h1_t[:, NPC:N], h1_nm, 8)
                    psn0 = pp.tile([128, 2 * GSL], F32, tag="gp")
                    psn1 = pp.tile([128, 2 * GSL], F32, tag="gp")
                    agg_emit(h1_nm, [psn0, psn1], range(16))
                    mean2_t = wk.tile([128, N], BF16, tag="m2", bufs=1)
                    h2_t = wk.tile([128, N], BF16, tag="o2", bufs=1)
                    for hf in range(2):
                        dsl = slice(NPC * hf, NPC * hf + NPC)
                        psum_drain([psn0, psn1][hf], mean2_t[:, dsl],
                                   AF.Identity)
                        psh = pp.tile([128, 2 * GSL], F32, tag="gp")
                        mm_halves(psh, [(w_r2_t[:, :], h1_t[:, dsl], 128),
                                        (w_l2_t[:, :], mean2_t[:, dsl], 128)])
                        psum_drain(psh, h2_t[:, dsl], AF.Identity,
                                   bias=b_l2_t[:, 0:1])

                # ---- head (full graph, in halves) ----
                pred = wk.tile([1, N], F32, tag="pred", bufs=1)
                for hf in range(2):
                    pso = pp.tile([1, 2 * GSL], F32, tag="ho", bufs=1)
                    for dc in range(2):
                        nc.tensor.matmul(
                            out=pso[0:1, GSL * dc:GSL * dc + CH],
                            lhsT=w_ou_t[:, 0:1],
                            rhs=h2_t[:, NPC * hf + CH * dc:
                                     NPC * hf + CH * dc + CH],
                            start=True, stop=True)
                    nc.vector.tensor_scalar(
                        out=pred[0:1, NPC * hf:NPC * hf + NPC].rearrange(
                            "p (c b) -> p c b", c=2),
                        in0=pso[0:1, :].rearrange("p (c b) -> p c b", c=2)
                        [:, :, 0:CH],
                        scalar1=b_ou_t[0:1, 0:1], scalar2=None, op0=AT.add)
                nc.sync.dma_start(out=out_d[0:1, :], in_=pred[0:1, :])
            pp_cm.__exit__(None, None, None)
            if rep_cm is not None:
                rep_cm.__exit__(None, None, None)

    return nc


def _prep_inputs(inputs, t_steps=T):
    """Host-side preprocessing: per-core input maps."""
    dyn = np.asarray(inputs["dynamic_features"], np.float32)
    sta = np.asarray(inputs["static_features"], np.float32)
    ei = np.asarray(inputs["edge_index"])
    W_ih = np.asarray(inputs["W_ih"], np.float32)
    W_hh = np.asarray(inputs["W_hh"], np.float32)
    b = (np.asarray(inputs["b_ih"], np.float32)
         + np.asarray(inputs["b_hh"], np.float32))
    W_sta = np.asarray(inputs["W_sta"], np.float32)
    b_sta = np.asarray(inputs["b_sta"], np.float32)
    W_fuse = np.asarray(inputs["W_fuse"], np.float32)
    b_fuse = np.asarray(inputs["b_fuse"], np.float32)
    s1_Wl = np.asarray(inputs["sage1_Wl"], np.float32)
    s1_bl = np.asarray(inputs["sage1_bl"], np.float32)
    s1_Wr = np.asarray(inputs["sage1_Wr"], np.float32)
    s2_Wl = np.asarray(inputs["sage2_Wl"], np.float32)
    s2_bl = np.asarray(inputs["sage2_bl"], np.float32)
    s2_Wr = np.asarray(inputs["sage2_Wr"], np.float32)
    W_out = np.asarray(inputs["W_out"], np.float32)
    b_out = np.asarray(inputs["b_out"], np.float32)

    tb = t_steps // 3

    # gate order in psum: [i, f, o, g]; torch order in weights: i,f,g,o
    gsl = [slice(0, H), slice(H, 2 * H), slice(3 * H, 4 * H), slice(2 * H, 3 * H)]
    # w_rec: lhsT [h_in, 4H]; g-gate 2x (tanh via sigmoid of doubled preact)
    w_rec_f32 = np.concatenate(
        [W_hh[gsl[0]].T, W_hh[gsl[1]].T,
         W_hh[gsl[2]].T, 2.0 * W_hh[gsl[3]].T], axis=1)
    w_rec = w_rec_f32.astype(BFNP)
    # w_x: [9, 4H] = [W_ih^T; bias row], g-gate 2x; replicated at 4x32 partitions
    wx9 = np.zeros((9, 4 * H), np.float32)
    for j, s in enumerate(gsl):
        sc = 2.0 if j == 3 else 1.0
        wx9[0:8, H * j:H * j + H] = sc * W_ih[s].T
        wx9[8, H * j:H * j + H] = sc * b[s]
    w_x = np.zeros((128, 4 * H), BFNP)
    for g in range(3):
        w_x[32 * g:32 * g + 9] = wx9.astype(BFNP)

    # fp8 fused DoubleRow weights: per x-group g: [W_hh | W_x-rows-at-32g]
    w_f = np.zeros((128, 3 * 2 * 4 * H), np.float32)
    for g in range(3):
        w_f[:, g * 1024:g * 1024 + 512] = w_rec_f32
        w_f[32 * g:32 * g + 9, g * 1024 + 512:g * 1024 + 1024] = wx9
    w_f = w_f.astype(E4NP)

    w_sta_t = np.zeros((F_STA + 1, H), np.float32)
    w_sta_t[0:F_STA] = W_sta.T
    w_sta_t[F_STA] = b_sta

    w_fz = np.ascontiguousarray(W_fuse[:, :H].T.astype(BFNP))
    w_fs = np.ascontiguousarray(W_fuse[:, H:].T)

    # normalized adjacency (same graph for every batch element)
    src, dst = ei[0].astype(np.int64), ei[1].astype(np.int64)
    cnt = np.bincount(dst, minlength=N).astype(np.float32)
    A = np.zeros((N, N), np.float32)
    np.add.at(A, (src, dst), 1.0)
    A /= np.maximum(cnt, 1.0)[None, :]
    A16 = np.ascontiguousarray(A).astype(BFNP)
    # odd cores aggregate in [own|peer] node order: permute rows+cols
    Pm = np.concatenate([np.arange(NPC, N), np.arange(0, NPC)])
    A16p = np.ascontiguousarray(A[Pm][:, Pm]).astype(BFNP)

    x_bn = dyn.transpose(0, 2, 1, 3).reshape(B * N, dyn.shape[1], F_DYN)
    sta_bn = sta.reshape(B * N, F_STA)

    shared = dict(
        w_rec=w_rec, w_x=w_x, w_f=w_f, w_sta=w_sta_t,
        w_fz=w_fz, w_fs=np.ascontiguousarray(W_fuse[:, H:].T.astype(BFNP)),
        b_fu=b_fuse.reshape(H, 1),
        w_r1=np.ascontiguousarray(s1_Wr.T.astype(BFNP)),
        w_l1=np.ascontiguousarray(s1_Wl.T.astype(BFNP)),
        b_l1=s1_bl.reshape(H, 1),
        w_r2=np.ascontiguousarray(s2_Wr.T.astype(BFNP)),
        w_l2=np.ascontiguousarray(s2_Wl.T.astype(BFNP)),
        b_l2=s2_bl.reshape(H, 1),
        w_ou=np.ascontiguousarray(W_out.T.astype(BFNP)),
        b_ou=b_out.reshape(1, 1),
    )

    in_maps = []
    for core in range(N_CORES):
        rows = slice(NPC * core, NPC * core + NPC)
        xc = x_bn[rows, 0:t_steps, :]                       # [NPC, T, 8]
        xt = xc.transpose(1, 2, 0)                          # [T, 8, NPC]
        arr = np.ones((tb, 3, 9, NPC), np.float32)
        arr[:, :, 0:8, :] = xt.reshape(tb, 3, 8, NPC)
        x_in = np.ascontiguousarray(arr.reshape(tb * 27, NPC)).astype(BFNP)
        arr8 = np.zeros((t_steps, 128, NPC), np.float32)
        a9 = arr.reshape(t_steps, 9, NPC)
        for g in range(3):
            arr8[g::3, 32 * g:32 * g + 9, :] = a9[g::3]
        x8_in = np.ascontiguousarray(
            arr8.reshape(t_steps * 128, NPC)).astype(E4NP)

        sta_in = np.ones((F_STA + 1, NPC), np.float32)
        sta_in[0:F_STA] = sta_bn[rows].T

        m = dict(shared)
        m.update(x_dyn=x_in, x8=x8_in, sta_t=sta_in,
                 a_mat=A16 if core % 2 == 0 else A16p)
        in_maps.append(m)
    return in_maps


def kernel(**inputs):
    t_steps = int(np.asarray(inputs["dynamic_features"]).shape[1])
    if t_steps not in _PROG_CACHE:
        nc_new = _build_program(t_steps)
        if not nc_new.is_finalized():
            nc_new.finalize()
        _PROG_CACHE[t_steps] = nc_new
    nc = _PROG_CACHE[t_steps]
    in_maps = _prep_inputs(inputs, t_steps)
    br = run_bass_kernel_spmd(nc, in_maps, list(range(N_CORES)),
                              trace=TRACE, **TRACE_KW)
    kernel.last_result = br
    # each pair core holds the full graph prediction; take the even cores'
    out = np.stack(
        [np.asarray(br.results[2 * g]["out"]).reshape(N) for g in range(B)])
    return out.astype(np.float32)



# revision 72
# speedup vs baseline: 75.2522x; 1.0132x over previous
"""Trainium2 Bass kernel for CombinedLSTMWithStatic2Hop.

Model: per-node LSTM over T timesteps + static encoder -> fusion -> 2x SAGEConv
(mean aggregation) -> linear head.

Sharding: B*N = 8000 nodes split into 1000 contiguous nodes per core (8 cores).
Each graph (2000 nodes) spans a core pair; SAGE aggregation uses pairwise
AllGather of node features between the two halves.

LSTM layout: hidden/gate dim on partitions, nodes on the free dim, node
chunks ping-ponging over 2x4 PSUM banks.  Gate nonlinearities use ONE
sigmoid ACT op over the 4 strided slots [i|f|o|g]; the g slot holds 2*a_g
(host-side 2x weight scale) so sigmoid gives Sg=(tanh(a_g)+1)/2:
  PSUM = W_hh[g] @ h + W_ih[g] @ x_t + b[g]     (g slot scaled 2x)
  S  = sigmoid(PSUM)             one ACT op per chunk  [i|f|o|g]
  Tg = 2*Sg - 1                  (DVE tensor_scalar, 4x rate)
  t2 = Si * Tg                   (DVE tensor_tensor, 2x rate)
  t1 = Sf * c                    (DVE tensor_tensor)
  c' = t1 + t2                   (DVE tensor_tensor)
  Tc = tanh(c')                  (ACT)
  h' = So * Tc                   (DVE tensor_tensor)
All state fp16 (DVE 2x/4x perf modes need 2-byte packed SBUF operands).
Biases enter through a ones-row on the x-side matmul (K=9).
"""

import os as _os

import ml_dtypes
import numpy as np

BFNP = np.float16

import concourse.bass as bass
import concourse.tile as tile
from concourse import bacc, mybir
from concourse.bass_utils import run_bass_kernel_spmd
from concourse.masks import make_identity

F32 = mybir.dt.float32
BF16 = mybir.dt.float16  # fp16: same PE rate as bf16, 8x finer mantissa
E4 = mybir.dt.float8e4   # fp8 e4m3: DoubleRow matmul at 2 cols/cycle
E4NP = ml_dtypes.float8_e4m3fn
FP8 = _os.environ.get("K_FP8", "1") != "0"

B, T, N, E = 4, 96, 2000, 16000
F_DYN, F_STA, H = 8, 16, 128
N_CORES = 8
NPC = B * N // N_CORES      # 1000 nodes per core
CH = NPC // 2               # 500 node chunk
GSL = 512                   # psum gate slot stride (one bank)

# module-level knobs (test.py may override)
TRACE = False
TRACE_KW = {}

_PROG_CACHE = {}


def _build_program(t_steps=T, repeat=1, gnn_repeat=1):
    nc = bacc.Bacc("TRN2", target_bir_lowering=False, debug=False,
                   num_devices=N_CORES)

    assert t_steps % 3 == 0
    tb = t_steps // 3  # x blocks of 3 timesteps (partition groups 0/32/64)

    # ---- DRAM I/O ----
    x_dram = nc.dram_tensor("x_dyn", [tb * 27, NPC], BF16, kind="ExternalInput")
    w_rec = nc.dram_tensor("w_rec", [H, 4 * H], BF16, kind="ExternalInput")
    w_x = nc.dram_tensor("w_x", [128, 4 * H], BF16, kind="ExternalInput")
    # fp8 fused path: per-t full-partition x images (zeros outside the
    # active 9-row group) + [W_hh | W_x-group] DoubleRow weights
    x8_dram = nc.dram_tensor("x8", [t_steps * 128, NPC], E4,
                             kind="ExternalInput")
    w_f = nc.dram_tensor("w_f", [128, 3 * 2 * 4 * H], E4,
                         kind="ExternalInput")
    w_sta = nc.dram_tensor("w_sta", [F_STA + 1, H], F32, kind="ExternalInput")
    sta_t = nc.dram_tensor("sta_t", [F_STA + 1, NPC], F32, kind="ExternalInput")
    w_fz = nc.dram_tensor("w_fz", [H, H], BF16, kind="ExternalInput")
    w_fs = nc.dram_tensor("w_fs", [H, H], BF16, kind="ExternalInput")
    b_fu = nc.dram_tensor("b_fu", [H, 1], F32, kind="ExternalInput")
    w_r1 = nc.dram_tensor("w_r1", [H, H], BF16, kind="ExternalInput")
    w_l1 = nc.dram_tensor("w_l1", [H, H], BF16, kind="ExternalInput")
    b_l1 = nc.dram_tensor("b_l1", [H, 1], F32, kind="ExternalInput")
    w_r2 = nc.dram_tensor("w_r2", [H, H], BF16, kind="ExternalInput")
    w_l2 = nc.dram_tensor("w_l2", [H, H], BF16, kind="ExternalInput")
    b_l2 = nc.dram_tensor("b_l2", [H, 1], F32, kind="ExternalInput")
    w_ou = nc.dram_tensor("w_ou", [H, 1], BF16, kind="ExternalInput")
    b_ou = nc.dram_tensor("b_ou", [1, 1], F32, kind="ExternalInput")
    a_mat = nc.dram_tensor("a_mat", [N, N], BF16, kind="ExternalInput")
    out_d = nc.dram_tensor("out", [1, NPC], F32, kind="ExternalOutput")

    AT = mybir.AluOpType
    AF = mybir.ActivationFunctionType
    n_kchunks = (N + 127) // 128  # 16 src chunks for aggregation

    with tile.TileContext(nc) as tc:
        with (
            tc.tile_pool(name="const", bufs=1) as cp,
            tc.tile_pool(name="xp", bufs=3) as xp,
            tc.tile_pool(name="wk", bufs=2) as wk,
        ):
            # ---- constants into SBUF ----
            def cload(dram, shape, tag, dt=F32):
                tl = cp.tile(shape, dt, tag=tag)
                nc.sync.dma_start(out=tl[:, :], in_=dram[:, :])
                return tl

            if FP8:
                # first on the queue: the t=0 critical path (x image +
                # fused weights) ahead of the other const loads
                xh_first = wk.tile([128, 2 * NPC], E4, tag="h",
                                   bufs=(4 if repeat > 1 else 3),
                                   name="xh0")
                nc.sync.dma_start(out=xh_first[:, NPC:2 * NPC],
                                  in_=x8_dram[0:128, :])
                nc.vector.memset(xh_first[:, 0:NPC], 0.0)
                w_f_t = cload(w_f, [128, 3 * 2 * 4 * H], "w_f", E4)
            else:
                w_rec_t = cload(w_rec, [H, 4 * H], "w_rec", BF16)
                w_x_t = cload(w_x, [128, 4 * H], "w_x", BF16)
            w_sta_tt = cload(w_sta, [F_STA + 1, H], "w_sta")
            sta_tt = cload(sta_t, [F_STA + 1, NPC], "sta_t")
            w_fz_t = cload(w_fz, [H, H], "w_fz", BF16)
            w_fs_t = cload(w_fs, [H, H], "w_fs", BF16)
            b_fu_t = cload(b_fu, [H, 1], "b_fu")
            w_r1_t = cload(w_r1, [H, H], "w_r1", BF16)
            w_l1_t = cload(w_l1, [H, H], "w_l1", BF16)
            b_l1_t = cload(b_l1, [H, 1], "b_l1")
            w_r2_t = cload(w_r2, [H, H], "w_r2", BF16)
            w_l2_t = cload(w_l2, [H, H], "w_l2", BF16)
            b_l2_t = cload(b_l2, [H, 1], "b_l2")
            w_ou_t = cload(w_ou, [H, 1], "w_ou", BF16)
            b_ou_t = cload(b_ou, [1, 1], "b_ou")

            ident = cp.tile([128, 128], BF16, tag="ident")
            make_identity(nc, ident[:, :])

            # 8MB of adjacency on the (idle) gpsimd DMA queue so it streams
            # during the LSTM instead of delaying the x loads / t=0
            a_tiles = []
            KC = N // 16  # 125-row src chunks over the FULL graph adjacency
            for k in range(16):
                tl = cp.tile([KC, N], BF16, tag=f"a{k}")
                nc.gpsimd.dma_start(out=tl[0:KC, :],
                                    in_=a_mat[KC * k:KC * k + KC, :])
                a_tiles.append(tl)

            # ---- LSTM ----
            # persistent psum gate tiles: [i|f|o|g] gate slots at 512 strides
            pl_cm = tc.tile_pool(name="psl", bufs=2, space="PSUM")
            pl = pl_cm.__enter__()

            # static encoder first: fills the ACT startup gap while the
            # first x DMAs land; reuses a gate psum buffer before the LSTM
            stl = wk.tile([128, NPC], BF16, tag="stl", bufs=1)
            pss = pl.tile([128, 4 * GSL], F32, tag="gates", name="pss")
            for dc in range(2):
                nc.tensor.matmul(
                    out=pss[:, GSL * dc:GSL * dc + CH],
                    lhsT=w_sta_tt[0:17, :],
                    rhs=sta_tt[0:17, CH * dc:CH * dc + CH],
                    start=True, stop=True)
            nc.scalar.activation(
                out=stl[:, :].rearrange("p (c b) -> p c b", c=2),
                in_=pss[:, 0:2 * GSL].rearrange("p (c b) -> p c b", c=2)
                [:, :, 0:CH],
                func=AF.Relu, scale=1.0)

            ps_ch = [pl.tile([128, 4 * GSL], F32, tag="gates", name="gates0"),
                     pl.tile([128, 4 * GSL], F32, tag="gates", name="gates1")]
            # zero the pad columns the gate ACT op reads ([500:512] of each slot)
            for ps in ps_ch:
                nc.vector.memset(ps[:, :], 0.0)

            rep_cm = tc.For_i(0, repeat, 1) if repeat > 1 else None
            if rep_cm is not None:
                rep_cm.__enter__()

            def load_xblock(b):
                tl = xp.tile([128, NPC], BF16, tag="xb")
                for g in range(3):
                    nc.sync.dma_start(
                        out=tl[32 * g:32 * g + 9, :],
                        in_=x_dram[27 * b + 9 * g:27 * b + 9 * g + 9, :])
                return tl

            CH_N = NPC // 4          # matmul chunk (250): psum write size
            PR = int(_os.environ.get("K_PRW", str(NPC // 4)))  # ACT/DVE width
            n_groups = NPC // PR
            mm_per_g = PR // CH_N
            TCW = int(_os.environ.get("K_TCW", str(NPC // 2)))  # Tc width
            tgr = TCW // PR          # groups per tail block
            h_bufs = (4 if repeat > 1 else 3) if FP8 else \
                     (3 if repeat > 1 else 2)

            if FP8:
                # combined [h | x] tiles: left half h(t), right half x(t+1).
                # Weight rows outside the active 9-row x group are zero, so
                # stale x data in other groups' rows cannot contaminate;
                # memset once so no row is ever read uninitialized.
                _xh_n = [0]

                def xh_alloc():
                    _xh_n[0] += 1
                    return wk.tile([128, 2 * NPC], E4, tag="h", bufs=h_bufs,
                                   name=f"xh{_xh_n[0]}")

                def dma_x8(tile, t):
                    nc.sync.dma_start(
                        out=tile[:, NPC:2 * NPC],
                        in_=x8_dram[128 * t:128 * t + 128, :])

                xh_prev = xh_first          # XH[-1]: h(-1)=0 | x(0)
                w_f_r = w_f_t[:, :].rearrange("p (x s m) -> p x s m",
                                              x=3, s=2)
            else:
                xtiles = {0: load_xblock(0)}
                if tb > 1:
                    xtiles[1] = load_xblock(1)

            s_prev = None
            h_prev = None
            for t in range(t_steps):
                blk, grp = divmod(t, 3)
                if not FP8:
                    if grp == 0 and blk + 2 < tb:
                        xtiles[blk + 2] = load_xblock(blk + 2)
                    xt = xtiles[blk]

                s_new = wk.tile([128, NPC], BF16, tag="s")
                if FP8:
                    xh_cur = xh_alloc()
                    if t + 1 < t_steps:
                        dma_x8(xh_cur, t + 1)
                    h_new = xh_cur
                    xh_rhs = xh_prev[:, :].rearrange("p (s n) -> p s n", s=2)
                else:
                    h_new = wk.tile([128, NPC], BF16, tag="h", bufs=h_bufs)
                tc_t = wk.tile([128, NPC], BF16, tag="tc", bufs=1)
                tts = {}
                tails = 0

                def emit_tail(b):
                    # Tc = tanh(c') for tail-block b (TCW cols), then
                    # h' = So * Tc per group (So lives in per-group tiles)
                    bsl = slice(TCW * b, TCW * b + TCW)
                    nc.scalar.activation(
                        out=tc_t[:, bsl], in_=s_new[:, bsl],
                        func=AF.Tanh, scale=1.0)
                    for pr in range(b * tgr, (b + 1) * tgr):
                        psl = slice(PR * pr, PR * pr + PR)
                        so = tts[pr][:, 2 * PR:3 * PR]
                        nc.vector.tensor_tensor(
                            out=h_new[:, psl], in0=so, in1=tc_t[:, psl],
                            op=AT.mult)

                for pr in range(n_groups):
                    base = PR * pr
                    ps = ps_ch[base // (NPC // 2)]
                    in_off = base % (NPC // 2)
                    psl = slice(base, base + PR)
                    tt = wk.tile([128, 4 * PR], BF16, tag=f"T{pr}",
                                 name=f"tt{pr}")
                    tts[pr] = tt

                    # matmuls in 250-col chunks; a psum tile's groups are
                    # CONTIGUOUS in each 512-col gate slot (off 0/250) so
                    # one strided ACT op covers a whole group
                    for c2 in range(mm_per_g):
                        sl = slice(base + CH_N * c2, base + CH_N * c2 + CH_N)
                        po = in_off + CH_N * c2
                        for gi in range(4):
                            osl = slice(GSL * gi + po, GSL * gi + po + CH_N)
                            if FP8:
                                # one DoubleRow mm: W_hh@h + W_xg@x, 2 col/cyc
                                nc.tensor.matmul(
                                    out=ps[:, osl],
                                    lhsT=w_f_r[:, grp, :,
                                               H * gi:H * gi + H],
                                    rhs=xh_rhs[:, :, sl],
                                    start=True, stop=True,
                                    perf_mode=mybir.MatmulPerfMode.DoubleRow)
                            else:
                                nc.tensor.matmul(
                                    out=ps[:, osl],
                                    lhsT=w_x_t[32 * grp:32 * grp + 9,
                                               H * gi:H * gi + H],
                                    rhs=xt[32 * grp:32 * grp + 9, sl],
                                    start=True, stop=(t == 0))
                        if t > 0 and not FP8:
                            for gi in range(4):
                                osl = slice(GSL * gi + po,
                                            GSL * gi + po + CH_N)
                                nc.tensor.matmul(
                                    out=ps[:, osl],
                                    lhsT=w_rec_t[:, H * gi:H * gi + H],
                                    rhs=h_prev[:, sl],
                                    start=False, stop=True)

                    # S = sigmoid(psum): one op, 4 strided slots x PR
                    nc.scalar.activation(
                        out=tt[:, :].rearrange("p (g b) -> p g b", g=4),
                        in_=ps[:, :].rearrange("p (g b) -> p g b", g=4)
                        [:, :, in_off:in_off + PR],
                        func=AF.Sigmoid, scale=1.0)

                    si = tt[:, 0:PR]
                    sf = tt[:, PR:2 * PR]
                    sg = tt[:, 3 * PR:4 * PR]
                    # Tg = 2*Sg - 1  (tensor_scalar, 4x)
                    tg = wk.tile([128, PR], BF16, tag=f"g{pr}",
                                 name=f"tg{pr}")
                    nc.vector.tensor_scalar(
                        out=tg[:, :], in0=sg, scalar1=2.0, scalar2=-1.0,
                        op0=AT.mult, op1=AT.add)
                    if t > 0:
                        t1 = wk.tile([128, PR], BF16, tag=f"q{pr}",
                                     name=f"t1{pr}")
                        t2 = wk.tile([128, PR], BF16, tag=f"p{pr}",
                                     name=f"t2{pr}")
                        # t2 = Si*Tg ; t1 = Sf*c ; c' = t1 + t2  (TT, 2x)
                        t1_eng = (nc.gpsimd if _os.environ.get("K_T1_POOL")
                                  else nc.vector)
                        t1_eng.tensor_tensor(
                            out=t1[:, :], in0=sf, in1=s_prev[:, psl],
                            op=AT.mult)
                        nc.vector.tensor_tensor(
                            out=t2[:, :], in0=si, in1=tg[:, :], op=AT.mult)
                        nc.vector.tensor_tensor(
                            out=s_new[:, psl], in0=t1[:, :], in1=t2[:, :],
                            op=AT.add)
                    else:
                        # c0 = Si*Tg
                        nc.vector.tensor_tensor(
                            out=s_new[:, psl], in0=si, in1=tg[:, :],
                            op=AT.mult)

                    # software pipeline: a tail block's tanh(c')/h goes
                    # after the NEXT group's gate ACT, so Tc's DVE wait
                    # doesn't block gate ops in the in-order ACT queue
                    while (tails + 1) * tgr <= pr:
                        emit_tail(tails)
                        tails += 1
                while tails * tgr < n_groups:
                    emit_tail(tails)
                    tails += 1

                s_prev, h_prev = s_new, h_new
                if FP8:
                    xh_prev = xh_cur

            if FP8:
                # cast the final h (fp8) to fp16 for the fusion matmul
                hh = wk.tile([128, NPC], BF16, tag="hh16", bufs=1)
                nc.vector.tensor_copy(out=hh[:, :], in_=h_prev[:, 0:NPC])
            else:
                hh = h_prev  # [128, NPC] = h_final
            # repeat (device For_i) wraps ONLY the collective-free LSTM;
            # the GNN phase is python-unrolled via gnn_repeat instead
            # (collectives inside a hardware loop wedge the runtime).
            if rep_cm is not None:
                rep_cm.__exit__(None, None, None)
                rep_cm = None
            if _os.environ.get("K_SKIP_GNN"):
                pred0 = wk.tile([1, NPC], F32, tag="pred0", bufs=1)
                nc.vector.tensor_copy(out=pred0[0:1, :], in_=hh[0:1, :])
                nc.sync.dma_start(out=out_d[0:1, :], in_=pred0[0:1, :])
                pl_cm.__exit__(None, None, None)
                return nc
            pl_cm.__exit__(None, None, None)
            pp_cm = tc.tile_pool(name="psg", bufs=2, space="PSUM")
            pp = pp_cm.__enter__()

            # ---- static encoder + fusion (own NPC nodes) ----
            def mm_halves(psum, pairs, width=NPC):
                # pairs: (lhsT_ap, rhs_tile, rhs_partitions); 500-col slots
                # at GSL strides (psum accumulation groups stay in-bank)
                nh = width // CH
                for c in range(nh):
                    osl = slice(GSL * c, GSL * c + CH)
                    for j, (lt, rtile, pr) in enumerate(pairs):
                        nc.tensor.matmul(
                            out=psum[:, osl], lhsT=lt,
                            rhs=rtile[0:pr, CH * c:CH * c + CH],
                            start=(j == 0), stop=(j == len(pairs) - 1))

            def psum_drain(psum, dst, func, bias=0.0, width=NPC, rows=128):
                # one strided ACT op over the GSL-strided slots
                nh = width // CH
                nc.scalar.activation(
                    out=dst[0:rows, 0:width].rearrange("p (c b) -> p c b", c=nh),
                    in_=psum[0:rows, 0:GSL * nh].rearrange(
                        "p (c b) -> p c b", c=nh)[:, :, 0:CH],
                    func=func, bias=bias, scale=1.0)

            node_t = wk.tile([128, NPC], BF16, tag="node", bufs=1)
            psf = pp.tile([128, 2 * GSL], F32, tag="gp")
            mm_halves(psf, [(w_fz_t[:, :], hh, 128), (w_fs_t[:, :], stl, 128)])
            psum_drain(psf, node_t, AF.Relu, bias=b_fu_t[:, 0:1])

            # ---- single pairwise exchange of fused node features ----
            # All aggregation/dst ordering is [own | peer] per core (host
            # permutes odd cores' adjacency to match); even cores' order is
            # global and only their outputs are read back.
            with tc.tile_pool(name="dram", bufs=1, space="DRAM") as dp:
                cc_in = dp.tile([128, NPC], BF16, tag="ci")
                cc_out = dp.tile([256, NPC], BF16, tag="co")
                nc.sync.dma_start(out=cc_in[:, :], in_=node_t[:, :])
                nc.gpsimd.collective_compute(
                    "AllGather", AT.bypass,
                    replica_groups=[[0, 1], [2, 3], [4, 5], [6, 7]],
                    ins=[cc_in.opt()], outs=[cc_out.opt()])

                def trans_chunks(x_t_tile, x_nm, k0):
                    # [128, NPC] feature-major -> 8 chunks [KC, 128] at k0..
                    for k in range(8):
                        trp = pp.tile([128, 128], BF16, tag="tr")
                        nc.tensor.transpose(
                            out=trp[0:KC, :],
                            in_=x_t_tile[:, KC * k:KC * k + KC],
                            identity=ident[:, :])
                        nc.vector.tensor_copy(
                            out=x_nm[0:KC, 128 * (k0 + k):128 * (k0 + k) + 128],
                            in_=trp[0:KC, :])

                def agg_emit(x_nm, psms, ks):
                    for k in ks:
                        for hf in range(2):
                            for dc in range(2):
                                osl = slice(GSL * dc, GSL * dc + CH)
                                nc.tensor.matmul(
                                    out=psms[hf][:, osl],
                                    lhsT=x_nm[0:KC, 128 * k:128 * k + 128],
                                    rhs=a_tiles[k][0:KC,
                                                   NPC * hf + CH * dc:
                                                   NPC * hf + CH * dc + CH],
                                    start=(k == 0), stop=(k == 15))

                for _gr in range(gnn_repeat):
                    # own-half transposes + own-src partial aggregation run
                    # UNDER the collective (they only need local node_t)
                    z_nm = wk.tile([128, 16 * 128], BF16, tag="znm", bufs=1)
                    trans_chunks(node_t, z_nm, 0)
                    psm0 = pp.tile([128, 2 * GSL], F32, tag="gp")
                    psm1 = pp.tile([128, 2 * GSL], F32, tag="gp")
                    agg_emit(z_nm, [psm0, psm1], range(0, 8))

                    # peer features: (block0 + block1) - node_t, symmetric
                    zb = wk.tile([128, 2 * NPC], BF16, tag="zb", bufs=1)
                    for r, eng in ((0, nc.sync), (1, nc.scalar)):
                        eng.dma_start(
                            out=zb[:, NPC * r:NPC * r + NPC],
                            in_=cc_out[128 * r:128 * r + 128, :])
                    z_peer = wk.tile([128, NPC], BF16, tag="zp", bufs=1)
                    nc.vector.tensor_tensor(
                        out=z_peer[:, :], in0=zb[:, 0:NPC],
                        in1=zb[:, NPC:2 * NPC], op=AT.add)
                    nc.vector.tensor_tensor(
                        out=z_peer[:, :], in0=z_peer[:, :],
                        in1=node_t[:, :], op=AT.subtract)
                    trans_chunks(z_peer, z_nm, 8)
                    agg_emit(z_nm, [psm0, psm1], range(8, 16))

                    mean_t = wk.tile([128, N], BF16, tag="m1", bufs=1)
                    h1_t = wk.tile([128, N], BF16, tag="o1", bufs=1)
                    roots = [node_t, z_peer]
                    for hf in range(2):
                        dsl = slice(NPC * hf, NPC * hf + NPC)
                        psum_drain([psm0, psm1][hf], mean_t[:, dsl],
                                   AF.Identity)
                        psh = pp.tile([128, 2 * GSL], F32, tag="gp")
                        mm_halves(psh, [(w_r1_t[:, :], roots[hf], 128),
                                        (w_l1_t[:, :], mean_t[:, dsl], 128)])
                        psum_drain(psh, h1_t[:, dsl], AF.Relu,
                                   bias=b_l1_t[:, 0:1])

                    # ---- SAGE 2: own-half dst only (host stitches the
                    # pair's halves back together) ----
                    h1_nm = wk.tile([128, 16 * 128], BF16, tag="hnm", bufs=1)
                    trans_chunks(h1_t[:, 0:NPC], h1_nm, 0)
                    trans_chunks(h1_t[:, NPC:N], h1_nm, 8)
                    psn0 = pp.tile([128, 2 * GSL], F32, tag="gp")
                    for k in range(16):
                        for dc in range(2):
                            osl = slice(GSL * dc, GSL * dc + CH)
                            nc.tensor.matmul(
                                out=psn0[:, osl],
                                lhsT=h1_nm[0:KC, 128 * k:128 * k + 128],
                                rhs=a_tiles[k][0:KC, CH * dc:CH * dc + CH],
                                start=(k == 0), stop=(k == 15))
                    mean2_t = wk.tile([128, NPC], BF16, tag="m2", bufs=1)
                    h2_t = wk.tile([128, NPC], BF16, tag="o2", bufs=1)
                    psum_drain(psn0, mean2_t[:, :], AF.Identity)
                    psh = pp.tile([128, 2 * GSL], F32, tag="gp")
                    mm_halves(psh, [(w_r2_t[:, :], h1_t[:, 0:NPC], 128),
                                    (w_l2_t[:, :], mean2_t[:, :], 128)])
                    psum_drain(psh, h2_t[:, :], AF.Identity,
                               bias=b_l2_t[:, 0:1])

                # ---- head (own half) ----
                pred = wk.tile([1, NPC], F32, tag="pred", bufs=1)
                pso = pp.tile([1, 2 * GSL], F32, tag="ho", bufs=1)
                for dc in range(2):
                    nc.tensor.matmul(
                        out=pso[0:1, GSL * dc:GSL * dc + CH],
                        lhsT=w_ou_t[:, 0:1],
                        rhs=h2_t[:, CH * dc:CH * dc + CH],
                        start=True, stop=True)
                nc.vector.tensor_scalar(
                    out=pred[0:1, :].rearrange("p (c b) -> p c b", c=2),
                    in0=pso[0:1, :].rearrange("p (c b) -> p c b", c=2)
                    [:, :, 0:CH],
                    scalar1=b_ou_t[0:1, 0:1], scalar2=None, op0=AT.add)
                nc.sync.dma_start(out=out_d[0:1, :], in_=pred[0:1, :])
            pp_cm.__exit__(None, None, None)
            if rep_cm is not None:
                rep_cm.__exit__(None, None, None)

    return nc


def _prep_inputs(inputs, t_steps=T):
    """Host-side preprocessing: per-core input maps."""
    dyn = np.asarray(inputs["dynamic_features"], np.float32)
    sta = np.asarray(inputs["static_features"], np.float32)
    ei = np.asarray(inputs["edge_index"])
    W_ih = np.asarray(inputs["W_ih"], np.float32)
    W_hh = np.asarray(inputs["W_hh"], np.float32)
    b = (np.asarray(inputs["b_ih"], np.float32)
         + np.asarray(inputs["b_hh"], np.float32))
    W_sta = np.asarray(inputs["W_sta"], np.float32)
    b_sta = np.asarray(inputs["b_sta"], np.float32)
    W_fuse = np.asarray(inputs["W_fuse"], np.float32)
    b_fuse = np.asarray(inputs["b_fuse"], np.float32)
    s1_Wl = np.asarray(inputs["sage1_Wl"], np.float32)
    s1_bl = np.asarray(inputs["sage1_bl"], np.float32)
    s1_Wr = np.asarray(inputs["sage1_Wr"], np.float32)
    s2_Wl = np.asarray(inputs["sage2_Wl"], np.float32)
    s2_bl = np.asarray(inputs["sage2_bl"], np.float32)
    s2_Wr = np.asarray(inputs["sage2_Wr"], np.float32)
    W_out = np.asarray(inputs["W_out"], np.float32)
    b_out = np.asarray(inputs["b_out"], np.float32)

    tb = t_steps // 3

    # gate order in psum: [i, f, o, g]; torch order in weights: i,f,g,o
    gsl = [slice(0, H), slice(H, 2 * H), slice(3 * H, 4 * H), slice(2 * H, 3 * H)]
    # w_rec: lhsT [h_in, 4H]; g-gate 2x (tanh via sigmoid of doubled preact)
    w_rec_f32 = np.concatenate(
        [W_hh[gsl[0]].T, W_hh[gsl[1]].T,
         W_hh[gsl[2]].T, 2.0 * W_hh[gsl[3]].T], axis=1)
    w_rec = w_rec_f32.astype(BFNP)
    # w_x: [9, 4H] = [W_ih^T; bias row], g-gate 2x; replicated at 4x32 partitions
    wx9 = np.zeros((9, 4 * H), np.float32)
    for j, s in enumerate(gsl):
        sc = 2.0 if j == 3 else 1.0
        wx9[0:8, H * j:H * j + H] = sc * W_ih[s].T
        wx9[8, H * j:H * j + H] = sc * b[s]
    w_x = np.zeros((128, 4 * H), BFNP)
    for g in range(3):
        w_x[32 * g:32 * g + 9] = wx9.astype(BFNP)

    # fp8 fused DoubleRow weights: per x-group g: [W_hh | W_x-rows-at-32g]
    w_f = np.zeros((128, 3 * 2 * 4 * H), np.float32)
    for g in range(3):
        w_f[:, g * 1024:g * 1024 + 512] = w_rec_f32
        w_f[32 * g:32 * g + 9, g * 1024 + 512:g * 1024 + 1024] = wx9
    w_f = w_f.astype(E4NP)

    w_sta_t = np.zeros((F_STA + 1, H), np.float32)
    w_sta_t[0:F_STA] = W_sta.T
    w_sta_t[F_STA] = b_sta

    w_fz = np.ascontiguousarray(W_fuse[:, :H].T.astype(BFNP))
    w_fs = np.ascontiguousarray(W_fuse[:, H:].T)

    # normalized adjacency (same graph for every batch element)
    src, dst = ei[0].astype(np.int64), ei[1].astype(np.int64)
    cnt = np.bincount(dst, minlength=N).astype(np.float32)
    A = np.zeros((N, N), np.float32)
    np.add.at(A, (src, dst), 1.0)
    A /= np.maximum(cnt, 1.0)[None, :]
    A16 = np.ascontiguousarray(A).astype(BFNP)
    # odd cores aggregate in [own|peer] node order: permute rows+cols
    Pm = np.concatenate([np.arange(NPC, N), np.arange(0, NPC)])
    A16p = np.ascontiguousarray(A[Pm][:, Pm]).astype(BFNP)

    x_bn = dyn.transpose(0, 2, 1, 3).reshape(B * N, dyn.shape[1], F_DYN)
    sta_bn = sta.reshape(B * N, F_STA)

    shared = dict(
        w_rec=w_rec, w_x=w_x, w_f=w_f, w_sta=w_sta_t,
        w_fz=w_fz, w_fs=np.ascontiguousarray(W_fuse[:, H:].T.astype(BFNP)),
        b_fu=b_fuse.reshape(H, 1),
        w_r1=np.ascontiguousarray(s1_Wr.T.astype(BFNP)),
        w_l1=np.ascontiguousarray(s1_Wl.T.astype(BFNP)),
        b_l1=s1_bl.reshape(H, 1),
        w_r2=np.ascontiguousarray(s2_Wr.T.astype(BFNP)),
        w_l2=np.ascontiguousarray(s2_Wl.T.astype(BFNP)),
        b_l2=s2_bl.reshape(H, 1),
        w_ou=np.ascontiguousarray(W_out.T.astype(BFNP)),
        b_ou=b_out.reshape(1, 1),
    )

    in_maps = []
    for core in range(N_CORES):
        rows = slice(NPC * core, NPC * core + NPC)
        xc = x_bn[rows, 0:t_steps, :]                       # [NPC, T, 8]
        xt = xc.transpose(1, 2, 0)                          # [T, 8, NPC]
        arr = np.ones((tb, 3, 9, NPC), np.float32)
        arr[:, :, 0:8, :] = xt.reshape(tb, 3, 8, NPC)
        x_in = np.ascontiguousarray(arr.reshape(tb * 27, NPC)).astype(BFNP)
        arr8 = np.zeros((t_steps, 128, NPC), np.float32)
        a9 = arr.reshape(t_steps, 9, NPC)
        for g in range(3):
            arr8[g::3, 32 * g:32 * g + 9, :] = a9[g::3]
        x8_in = np.ascontiguousarray(
            arr8.reshape(t_steps * 128, NPC)).astype(E4NP)

        sta_in = np.ones((F_STA + 1, NPC), np.float32)
        sta_in[0:F_STA] = sta_bn[rows].T

        m = dict(shared)
        m.update(x_dyn=x_in, x8=x8_in, sta_t=sta_in,
                 a_mat=A16 if core % 2 == 0 else A16p)
        in_maps.append(m)
    return in_maps


def kernel(**inputs):
    t_steps = int(np.asarray(inputs["dynamic_features"]).shape[1])
    if t_steps not in _PROG_CACHE:
        nc_new = _build_program(t_steps)
        if not nc_new.is_finalized():
            nc_new.finalize()
        _PROG_CACHE[t_steps] = nc_new
    nc = _PROG_CACHE[t_steps]
    in_maps = _prep_inputs(inputs, t_steps)
    br = run_bass_kernel_spmd(nc, in_maps, list(range(N_CORES)),
                              trace=TRACE, **TRACE_KW)
    kernel.last_result = br
    # each core predicts its own NPC nodes (odd cores' [own|peer] dst
    # ordering puts "own" first, so core c always yields global rows
    # [NPC*c : NPC*c+NPC])
    out = np.concatenate(
        [np.asarray(br.results[c]["out"]).reshape(NPC) for c in range(N_CORES)])
    return out.reshape(B, N).astype(np.float32)

